# revision 8
# baseline (speedup 1.0000x reference)
"""HQQ 4-bit quantized linear on 8 Trainium2 NeuronCores (Bass/Tile).

out[4096, 11008] = x[4096, 4096] @ dequant(W_q, scale, zero).T + bias

Key index fact: reference reshapes ((W_r - zero) * scale) from [64, 704512]
to [11008, 4096].  With o = output feature, i = input feature:
    o = g_row * 172 + j,   group g = j * 4096 + i,   g_row in [0, 64)
so sharding 8 consecutive g_rows per core gives each core a contiguous
1376-column output slice (column-parallel linear, x replicated).

Per-core pipeline:
  phase 1: DMA W_q rows + scale/zero, extract nibble, dequantize to fp16,
           PE-transpose into a resident [4096(i) x 1376(o)] fp16 W.T
  phase 2: stream x row-blocks, convert fp16, PE-transpose to x.T tiles,
           accumulate out[t-tile, o-tile] = sum_k x.T[k,t].T @ W.T[k,o]
           in PSUM (bias pre-loaded via a K=1 ones x bias matmul).
"""

import numpy as np
from contextlib import ExitStack

import concourse.bacc as bacc
import concourse.bass as bass
import concourse.mybir as mybir
import concourse.tile as tile
from concourse.bass_utils import run_bass_kernel_spmd

dt = mybir.dt
Alu = mybir.AluOpType

TOKENS, IN_F, OUT_F, GS = 4096, 4096, 11008, 64
G = OUT_F * IN_F // GS            # 704512 quantization groups
J = G // IN_F                     # 172 groups per (g_row, i) plane
NCORES = 8
RPC = GS // NCORES                # 8 g_rows per core
O_C = RPC * J                     # 1376 output cols per core
NT = TOKENS // 128                # 32 token tiles
NK = IN_F // 128                  # 32 contraction blocks
NO = 344                          # psum o-tile width (<=512 fp32 psum bank)
NB = O_C // NO                    # 4 psum tiles per token tile
IC = 512                          # i-chunk for dequant / x streaming
JSPLIT = ((0, 128), (128, J - 128))   # j=172 -> partitions 128 + 44

_CACHE = {}


def _build():
    nc = bacc.Bacc("TRN2", target_bir_lowering=False, debug=False,
                   num_devices=NCORES)

    x_d = nc.dram_tensor("x", [TOKENS, IN_F], dt.float32, kind="ExternalInput")
    q_d = nc.dram_tensor("wq", [RPC, J, IN_F], dt.int32, kind="ExternalInput")
    s_d = nc.dram_tensor("scale", [J, IN_F], dt.float32, kind="ExternalInput")
    z_d = nc.dram_tensor("zero", [J, IN_F], dt.float32, kind="ExternalInput")
    b_d = nc.dram_tensor("bias", [1, O_C], dt.float32, kind="ExternalInput")
    hs_d = nc.dram_tensor("hsel", [128, 1], dt.float32, kind="ExternalInput")
    ls_d = nc.dram_tensor("lsel", [128, 1], dt.float32, kind="ExternalInput")
    id_d = nc.dram_tensor("ident", [128, 128], dt.float16, kind="ExternalInput")
    o_d = nc.dram_tensor("out", [TOKENS, O_C], dt.float32, kind="ExternalOutput")

    with ExitStack() as ctx:
        tc = ctx.enter_context(tile.TileContext(nc))
        const = ctx.enter_context(tc.tile_pool(name="const", bufs=1))
        ph1 = ctx.enter_context(tc.tile_pool(name="ph1", bufs=2))
        ph2 = ctx.enter_context(tc.tile_pool(name="ph2", bufs=3))
        opool = ctx.enter_context(tc.tile_pool(name="opool", bufs=4))
        pacc = ctx.enter_context(
            tc.tile_pool(name="pacc", bufs=1, space=bass.MemorySpace.PSUM))
        ptr = ctx.enter_context(
            tc.tile_pool(name="ptr", bufs=3, space=bass.MemorySpace.PSUM))

        ident = const.tile([128, 128], dt.float16)
        nc.sync.dma_start(ident[:], id_d[:])
        hsel = const.tile([128, 1], dt.float32)
        nc.sync.dma_start(hsel[:], hs_d[:])
        lsel = const.tile([128, 1], dt.float32)
        nc.sync.dma_start(lsel[:], ls_d[:])
        biasf = const.tile([1, O_C], dt.float32)
        nc.sync.dma_start(biasf[:], b_d[:])
        biash = const.tile([1, O_C], dt.float16)
        nc.scalar.copy(biash[:], biasf[:])
        ones = const.tile([1, 128], dt.float16)
        nc.vector.memset(ones[:], 1.0)

        # resident transposed dequantized weights: [i-partition, k-block, o]
        WT = const.tile([128, NK, O_C], dt.float16)

        # ---- phase 1: dequant + transpose W ----
        for ic in range(IN_F // IC):          # 8 chunks of 512 along i
            sz = {}
            for (jb, jn) in JSPLIT:
                st = ph1.tile([jn, IC], dt.float32, tag=f"s{jb}")
                zt = ph1.tile([jn, IC], dt.float32, tag=f"z{jb}")
                nc.sync.dma_start(st[:], s_d[jb:jb + jn, ic * IC:(ic + 1) * IC])
                nc.sync.dma_start(zt[:], z_d[jb:jb + jn, ic * IC:(ic + 1) * IC])
                zs = ph1.tile([jn, IC], dt.float32, tag=f"zs{jb}")
                nc.vector.tensor_mul(zs[:], zt[:], st[:])
                sz[jb] = (st, zs)
            for r in range(RPC):
                for (jb, jn) in JSPLIT:
                    st, zs = sz[jb]
                    q = ph1.tile([jn, IC], dt.int32, tag=f"q{jb}")
                    nc.sync.dma_start(
                        q[:], q_d[r, jb:jb + jn, ic * IC:(ic + 1) * IC])
                    hi_i = ph1.tile([jn, IC], dt.int32, tag=f"hi{jb}")
                    nc.vector.tensor_scalar(
                        hi_i[:], q[:], 4, 15,
                        Alu.logical_shift_right, Alu.bitwise_and)
                    lo_i = ph1.tile([jn, IC], dt.int32, tag=f"li{jb}")
                    nc.vector.tensor_single_scalar(
                        lo_i[:], q[:], 15, Alu.bitwise_and)
                    # nf = hi*hsel + lo*lsel  (exact 0/1 per-core select;
                    # arith tensor_scalar casts int32 -> fp32 on write)
                    hi = ph1.tile([jn, IC], dt.float32, tag=f"h{jb}")
                    nc.vector.tensor_scalar_mul(hi[:], hi_i[:], hsel[0:jn, 0:1])
                    lo = ph1.tile([jn, IC], dt.float32, tag=f"l{jb}")
                    nc.vector.tensor_scalar_mul(lo[:], lo_i[:], lsel[0:jn, 0:1])
                    nf = ph1.tile([jn, IC], dt.float32, tag=f"f{jb}")
                    nc.vector.tensor_add(nf[:], hi[:], lo[:])
                    wh = ph1.tile([jn, IC], dt.float16, tag=f"w{jb}")
                    nc.vector.tensor_mul(nf[:], nf[:], st[:])
                    nc.vector.tensor_sub(wh[:], nf[:], zs[:])  # fp16 out
                    for sub in range(IC // 128):
                        k = ic * (IC // 128) + sub
                        pt = ptr.tile([128, jn], dt.float16, tag="tr")
                        nc.tensor.transpose(
                            pt[:, 0:jn], wh[0:jn, sub * 128:(sub + 1) * 128],
                            ident[0:jn, 0:jn])
                        nc.scalar.copy(
                            WT[:, k, r * J + jb:r * J + jb + jn], pt[:, 0:jn])

        # ---- phase 2: stream x, transpose, matmul ----
        for t in range(NT):
            acc = []
            for p in range(NB):
                a = pacc.tile([128, NO], dt.float32, tag=f"a{p}")
                nc.tensor.matmul(
                    a[:], ones[0:1, :], biash[0:1, p * NO:(p + 1) * NO],
                    start=True, stop=False)
                acc.append(a)
            for icc in range(IN_F // IC):
                xr = ph2.tile([128, IC], dt.float32, tag="xr")
                nc.sync.dma_start(
                    xr[:], x_d[t * 128:(t + 1) * 128, icc * IC:(icc + 1) * IC])
                xh = ph2.tile([128, IC], dt.float16, tag="xh")
                nc.scalar.copy(xh[:], xr[:])
                for sub in range(IC // 128):
                    k = icc * (IC // 128) + sub
                    pt = ptr.tile([128, 128], dt.float16, tag="tr")
                    nc.tensor.transpose(
                        pt[:], xh[:, sub * 128:(sub + 1) * 128], ident[:])
                    xT = ph2.tile([128, 128], dt.float16, tag="xT")
                    nc.vector.tensor_copy(xT[:], pt[:])
                    for p in range(NB):
                        nc.tensor.matmul(
                            acc[p][:], xT[:], WT[:, k, p * NO:(p + 1) * NO],
                            start=False, stop=(k == NK - 1))
            for p in range(NB):
                ob = opool.tile([128, NO], dt.float32, tag=f"o{p}")
                nc.vector.tensor_copy(ob[:], acc[p][:])
                nc.sync.dma_start(
                    o_d[t * 128:(t + 1) * 128, p * NO:(p + 1) * NO], ob[:])

    nc.compile()
    return nc


def get_nc():
    if "nc" not in _CACHE:
        _CACHE["nc"] = _build()
    return _CACHE["nc"]


def make_in_maps(x, W_q, scale, zero, bias):
    x = np.ascontiguousarray(x, dtype=np.float32)
    W_q = np.ascontiguousarray(W_q, dtype=np.int32)
    s2 = np.ascontiguousarray(scale, dtype=np.float32).reshape(J, IN_F)
    z2 = np.ascontiguousarray(zero, dtype=np.float32).reshape(J, IN_F)
    bias = np.ascontiguousarray(bias, dtype=np.float32)
    ident = np.eye(128, dtype=np.float16)
    in_maps = []
    for c in range(NCORES):
        r0 = RPC * (c % 4)
        in_maps.append({
            "x": x,
            "wq": np.ascontiguousarray(W_q[r0:r0 + RPC]).reshape(RPC, J, IN_F),
            "scale": s2,
            "zero": z2,
            "bias": bias[c * O_C:(c + 1) * O_C].reshape(1, O_C),
            "hsel": np.full((128, 1), 1.0 if c < 4 else 0.0, dtype=np.float32),
            "lsel": np.full((128, 1), 0.0 if c < 4 else 1.0, dtype=np.float32),
            "ident": ident,
        })
    return in_maps


def kernel(x, W_q, scale, zero, bias):
    nc = get_nc()
    in_maps = make_in_maps(x, W_q, scale, zero, bias)
    res = run_bass_kernel_spmd(nc, in_maps, list(range(NCORES)))
    return np.concatenate(
        [res.results[c]["out"] for c in range(NCORES)], axis=1)


# revision 12
# speedup vs baseline: 1.1167x; 1.1167x over previous
"""HQQ 4-bit quantized linear on 8 Trainium2 NeuronCores (Bass/Tile).

out[4096, 11008] = x[4096, 4096] @ dequant(W_q, scale, zero).T + bias

Key index fact: reference reshapes ((W_r - zero) * scale) from [64, 704512]
to [11008, 4096].  With o = output feature, i = input feature:
    o = g_row * 172 + j,   group g = j * 4096 + i,   g_row in [0, 64)
so sharding 8 consecutive g_rows per core gives each core a contiguous
1376-column output slice (column-parallel linear, x replicated).

Per-core pipeline:
  phase 1: DMA W_q rows + scale/zero, extract nibble, dequantize to fp16,
           PE-transpose into a resident [4096(i) x 1376(o)] fp16 W.T
  phase 2: stream x row-blocks, convert fp16, PE-transpose to x.T tiles,
           accumulate out[t-tile, o-tile] = sum_k x.T[k,t].T @ W.T[k,o]
           in PSUM (bias pre-loaded via a K=1 ones x bias matmul).
"""

import numpy as np
from contextlib import ExitStack

import concourse.bacc as bacc
import concourse.bass as bass
import concourse.mybir as mybir
import concourse.tile as tile
from concourse.bass_utils import run_bass_kernel_spmd

dt = mybir.dt
Alu = mybir.AluOpType

TOKENS, IN_F, OUT_F, GS = 4096, 4096, 11008, 64
G = OUT_F * IN_F // GS            # 704512 quantization groups
J = G // IN_F                     # 172 groups per (g_row, i) plane
NCORES = 8
RPC = GS // NCORES                # 8 g_rows per core
O_C = RPC * J                     # 1376 output cols per core
NT = TOKENS // 128                # 32 token tiles
NK = IN_F // 128                  # 32 contraction blocks
NO = 344                          # psum o-tile width (<=512 fp32 psum bank)
NB = O_C // NO                    # 4 psum tiles per token tile
IC = 512                          # i-chunk for dequant / x streaming
JSPLIT = ((0, 128), (128, J - 128))   # j=172 -> partitions 128 + 44

_CACHE = {}


def _build():
    nc = bacc.Bacc("TRN2", target_bir_lowering=False, debug=False,
                   num_devices=NCORES)

    x_d = nc.dram_tensor("x", [TOKENS, IN_F], dt.float32, kind="ExternalInput")
    q_d = nc.dram_tensor("wq", [RPC, J, IN_F], dt.int32, kind="ExternalInput")
    s_d = nc.dram_tensor("scale", [J, IN_F], dt.float32, kind="ExternalInput")
    z_d = nc.dram_tensor("zero", [J, IN_F], dt.float32, kind="ExternalInput")
    b_d = nc.dram_tensor("bias", [1, O_C], dt.float32, kind="ExternalInput")
    hs_d = nc.dram_tensor("hsel", [128, 1], dt.float32, kind="ExternalInput")
    ls_d = nc.dram_tensor("lsel", [128, 1], dt.float32, kind="ExternalInput")
    id_d = nc.dram_tensor("ident", [128, 128], dt.float16, kind="ExternalInput")
    o_d = nc.dram_tensor("out", [TOKENS, O_C], dt.float32, kind="ExternalOutput")

    with ExitStack() as ctx:
        tc = ctx.enter_context(tile.TileContext(nc))
        const = ctx.enter_context(tc.tile_pool(name="const", bufs=1))
        ph1 = ctx.enter_context(tc.tile_pool(name="ph1", bufs=2))
        ph2 = ctx.enter_context(tc.tile_pool(name="ph2", bufs=8))
        xtp = ctx.enter_context(tc.tile_pool(name="xtp", bufs=16))
        opool = ctx.enter_context(tc.tile_pool(name="opool", bufs=4))
        pacc = ctx.enter_context(
            tc.tile_pool(name="pacc", bufs=1, space=bass.MemorySpace.PSUM))
        ptr = ctx.enter_context(
            tc.tile_pool(name="ptr", bufs=4, space=bass.MemorySpace.PSUM))

        ident = const.tile([128, 128], dt.float16)
        nc.sync.dma_start(ident[:], id_d[:])
        hsel = const.tile([128, 1], dt.float32)
        nc.sync.dma_start(hsel[:], hs_d[:])
        lsel = const.tile([128, 1], dt.float32)
        nc.sync.dma_start(lsel[:], ls_d[:])
        biasf = const.tile([1, O_C], dt.float32)
        nc.sync.dma_start(biasf[:], b_d[:])
        biash = const.tile([1, O_C], dt.float16)
        nc.scalar.copy(biash[:], biasf[:])
        ones = const.tile([1, 128], dt.float16)
        nc.vector.memset(ones[:], 1.0)

        # resident transposed dequantized weights: [i-partition, k-block, o]
        WT = const.tile([128, NK, O_C], dt.float16)

        # ---- phase 1: dequant + transpose W ----
        for ic in range(IN_F // IC):          # 8 chunks of 512 along i
            sz = {}
            for (jb, jn) in JSPLIT:
                st = ph1.tile([jn, IC], dt.float32, tag=f"s{jb}")
                zt = ph1.tile([jn, IC], dt.float32, tag=f"z{jb}")
                nc.sync.dma_start(st[:], s_d[jb:jb + jn, ic * IC:(ic + 1) * IC])
                nc.sync.dma_start(zt[:], z_d[jb:jb + jn, ic * IC:(ic + 1) * IC])
                zs = ph1.tile([jn, IC], dt.float32, tag=f"zs{jb}")
                nc.vector.tensor_mul(zs[:], zt[:], st[:])
                sz[jb] = (st, zs)
            for r in range(RPC):
                for (jb, jn) in JSPLIT:
                    st, zs = sz[jb]
                    q = ph1.tile([jn, IC], dt.int32, tag=f"q{jb}")
                    nc.sync.dma_start(
                        q[:], q_d[r, jb:jb + jn, ic * IC:(ic + 1) * IC])
                    # nib = hsel*((q - (q&15))/16) + lsel*(q&15)
                    #     = (hsel/16)*q + (lsel - hsel/16)*(q&15)
                    # with per-core constants A=hsel/16, B=lsel-hsel/16:
                    # exact in fp32 (q < 256), selects hi or lo nibble.
                    lo_i = ph1.tile([jn, IC], dt.int32, tag=f"li{jb}")
                    nc.vector.tensor_single_scalar(
                        lo_i[:], q[:], 15, Alu.bitwise_and)
                    t1 = ph1.tile([jn, IC], dt.float32, tag=f"t{jb}")
                    nc.vector.tensor_scalar_mul(t1[:], q[:], hsel[0:jn, 0:1])
                    nf = ph1.tile([jn, IC], dt.float32, tag=f"f{jb}")
                    nc.vector.scalar_tensor_tensor(
                        nf[:], lo_i[:], lsel[0:jn, 0:1], t1[:],
                        Alu.mult, Alu.add)
                    wh = ph1.tile([jn, IC], dt.float16, tag=f"w{jb}")
                    nc.vector.tensor_mul(nf[:], nf[:], st[:])
                    nc.vector.tensor_sub(wh[:], nf[:], zs[:])  # fp16 out
                    for sub in range(IC // 128):
                        k = ic * (IC // 128) + sub
                        pt = ptr.tile([128, jn], dt.float16, tag="tr")
                        nc.tensor.transpose(
                            pt[:, 0:jn], wh[0:jn, sub * 128:(sub + 1) * 128],
                            ident[0:jn, 0:jn])
                        nc.scalar.copy(
                            WT[:, k, r * J + jb:r * J + jb + jn], pt[:, 0:jn])

        # ---- phase 2: stream x, transpose, matmul ----
        for t in range(NT):
            acc = []
            for p in range(NB):
                a = pacc.tile([128, NO], dt.float32, tag=f"a{p}")
                nc.tensor.matmul(
                    a[:], ones[0:1, :], biash[0:1, p * NO:(p + 1) * NO],
                    start=True, stop=False)
                acc.append(a)
            for icc in range(IN_F // IC):
                xr = ph2.tile([128, IC], dt.float32, tag="xr")
                nc.sync.dma_start(
                    xr[:], x_d[t * 128:(t + 1) * 128, icc * IC:(icc + 1) * IC])
                xh = ph2.tile([128, IC], dt.float16, tag="xh")
                nc.scalar.copy(xh[:], xr[:])
                for sub in range(IC // 128):
                    k = icc * (IC // 128) + sub
                    pt = ptr.tile([128, 128], dt.float16, tag="tr")
                    nc.tensor.transpose(
                        pt[:], xh[:, sub * 128:(sub + 1) * 128], ident[:])
                    xT = xtp.tile([128, 128], dt.float16, tag="xT")
                    nc.vector.tensor_copy(xT[:], pt[:])
                    for p in range(NB):
                        nc.tensor.matmul(
                            acc[p][:], xT[:], WT[:, k, p * NO:(p + 1) * NO],
                            start=False, stop=(k == NK - 1))
            for p in range(NB):
                ob = opool.tile([128, NO], dt.float32, tag=f"o{p}")
                nc.vector.tensor_copy(ob[:], acc[p][:])
                nc.sync.dma_start(
                    o_d[t * 128:(t + 1) * 128, p * NO:(p + 1) * NO], ob[:])

    nc.compile()
    return nc


def get_nc():
    if "nc" not in _CACHE:
        _CACHE["nc"] = _build()
    return _CACHE["nc"]


def make_in_maps(x, W_q, scale, zero, bias):
    x = np.ascontiguousarray(x, dtype=np.float32)
    W_q = np.ascontiguousarray(W_q, dtype=np.int32)
    s2 = np.ascontiguousarray(scale, dtype=np.float32).reshape(J, IN_F)
    z2 = np.ascontiguousarray(zero, dtype=np.float32).reshape(J, IN_F)
    bias = np.ascontiguousarray(bias, dtype=np.float32)
    ident = np.eye(128, dtype=np.float16)
    in_maps = []
    for c in range(NCORES):
        r0 = RPC * (c % 4)
        in_maps.append({
            "x": x,
            "wq": np.ascontiguousarray(W_q[r0:r0 + RPC]).reshape(RPC, J, IN_F),
            "scale": s2,
            "zero": z2,
            "bias": bias[c * O_C:(c + 1) * O_C].reshape(1, O_C),
            "hsel": np.full((128, 1), 1.0 / 16 if c < 4 else 0.0,
                            dtype=np.float32),
            "lsel": np.full((128, 1), -1.0 / 16 if c < 4 else 1.0,
                            dtype=np.float32),
            "ident": ident,
        })
    return in_maps


def kernel(x, W_q, scale, zero, bias):
    nc = get_nc()
    in_maps = make_in_maps(x, W_q, scale, zero, bias)
    res = run_bass_kernel_spmd(nc, in_maps, list(range(NCORES)))
    return np.concatenate(
        [res.results[c]["out"] for c in range(NCORES)], axis=1)
